# revision 33
# baseline (speedup 1.0000x reference)
"""ACN2d multi-branch attentive normalization on 8 TRN2 NeuronCores.

Sharding: data-parallel over batch B (8 samples -> 8 cores), no collectives.
Per core, a two-phase Bass/Tile kernel:
  phase 1 (point-major): PE transposes x to fp16 point-major tiles and
    computes att^T; softmax over the G=8 groups is a free-dim reduce on DVE;
    weighted moments P = x a^T, Q = x^2 a^T and s = sum_n a accumulate on PE.
    a^T is also transposed back to group-major (ags_gm) here, using the
    otherwise-idle PE/ACT/DVE slack, so phase 2 is matmul-only.
  phase 2 (channel-major): A1/A2 are K=8 matmuls against inv_std^T /
    (mean inv_std)^T streaming group-major a; ACT stages A1 to SBUF f16 and
    DVE combines out = x*A1 - A2; results stream to HBM as f16.
The tensor-engine clock drops to a half-rate p-state on any long idle and
needs ~3us of continuous work to recover, so the emission order keeps PE
fed across the stats bubble (filler transposes) and phase 2 keeps >=1
iteration of slack on every cross-engine dependency.
"""

import numpy as np

from concourse.bass_utils import run_bass_kernel_spmd


from contextlib import ExitStack

import concourse.bass as bass
import concourse.bacc as bacc
import concourse.tile as tile
from concourse import mybir
from concourse.masks import make_identity

F32 = mybir.dt.float32
F16 = mybir.dt.float16
AF = mybir.ActivationFunctionType
OP = mybir.AluOpType
AX = mybir.AxisListType

EPS = 1e-3
A_NORM_EPS = 1e-8


def bcast_last(ap, n):
    return bass.AP(tensor=ap.tensor, offset=ap.offset, ap=list(ap.ap) + [[0, n]])


def build_nc(N=32768, fold_bias=True):
    C, G = 128, 8
    TP = 128
    GRP = 2048
    JJ = GRP // TP                # tiles per group = 16
    ntiles = N // TP
    ngrp = N // GRP
    assert N % GRP == 0

    nc = bacc.Bacc("TRN2", target_bir_lowering=False, debug=False)
    x_ext = nc.declare_dram_parameter("x", [C, N], F32, isOutput=False).ap()
    w_ext = nc.declare_dram_parameter("conv_w", [G, C], F32, isOutput=False).ap()
    b_ext = nc.declare_dram_parameter("conv_b", [1, G], F32, isOutput=False).ap()
    out_ext = nc.declare_dram_parameter("out", [C, N], F16, isOutput=True).ap()

    with tile.TileContext(nc) as tc, ExitStack() as ctx:
        consts = ctx.enter_context(tc.tile_pool(name="consts", bufs=1))
        resident = ctx.enter_context(tc.tile_pool(name="resident", bufs=1))
        stats = ctx.enter_context(tc.tile_pool(name="stats", bufs=1))
        # SBUF staging pools stay open for the whole kernel so later pools
        # never inherit WAW deps on the DMA lanes that filled earlier ones.
        xstage = ctx.enter_context(tc.tile_pool(name="xstage", bufs=2))
        x2stage = ctx.enter_context(tc.tile_pool(name="x2stage", bufs=2))
        estage = ctx.enter_context(tc.tile_pool(name="estage", bufs=2))
        zstage = ctx.enter_context(tc.tile_pool(name="zstage", bufs=2))
        a12_sbp = ctx.enter_context(tc.tile_pool(name="a12_sb", bufs=1))
        tmp_pool = ctx.enter_context(tc.tile_pool(name="tmp_sb", bufs=2))
        opool = ctx.enter_context(tc.tile_pool(name="ostage", bufs=1))
        xs0pool = ctx.enter_context(tc.tile_pool(name="xs0p", bufs=1))

        # first x chunk: issue its load before anything else, split in two
        # so the first cast/transposes start after half the transfer
        xs0 = xs0pool.tile([C, GRP], F32, name="xs0")
        nc.sync.dma_start(xs0[:, 0:GRP // 2], x_ext[:, 0:GRP // 2])
        nc.sync.dma_start(xs0[:, GRP // 2:GRP], x_ext[:, GRP // 2:GRP])

        # ---- constants: everything the PE touches funnels through gpsimd ----
        ident = consts.tile([128, 128], F16)
        make_identity(nc, ident)
        ident8 = consts.tile([G, G], F32)
        make_identity(nc, ident8)
        onesf32 = consts.tile([128, 1], F32)
        nc.vector.memset(onesf32, 1.0)
        eps_t = consts.tile([G, 1], F32)
        nc.vector.memset(eps_t, EPS)
        w_sb = consts.tile([G, C], F32)
        nc.sync.dma_start(w_sb[:], w_ext)
        b_row = consts.tile([1, G], F32)
        nc.sync.dma_start(b_row[:], b_ext)
        ones_col = consts.tile([1, 128], F16)
        nc.gpsimd.memset(ones_col, 1.0)
        b_rep = consts.tile([1, JJ, G], F16)
        nc.gpsimd.tensor_copy(
            b_rep[:],
            bass.AP(tensor=b_row[:].tensor, offset=b_row[:].offset,
                    ap=[b_row[:].ap[0], [0, JJ], b_row[:].ap[1]]))
        # bounce conv_w through gpsimd so the wT transpose waits on Pool only
        w_g = consts.tile([G, C], F32)
        nc.gpsimd.tensor_copy(w_g[:], w_sb[:])

        with tc.tile_pool(name="ph0psum", bufs=1, space="PSUM") as ph0psum:
            wT_ps = ph0psum.tile([C, G], F32)
            nc.tensor.transpose(wT_ps[:], w_g[:], ident8[:])
            wT = consts.tile([C, G], F16)
            nc.scalar.copy(wT[:], wT_ps[:])

        # ---- residents ----
        xc = resident.tile([C, N], F16)
        # x^T ring buffer: only the previous group's transpose is ever read
        xtr = resident.tile([128, 2, GRP], F16)
        a_t = resident.tile([128, N // 16], F16)
        # group-major a, built incrementally during phase 1
        ags_gm = resident.tile([G, N], F16)
        # per-group partial sums of a (reduced over tiles); folded into a
        # single cheap reduce at stats time instead of one 3.8us monster.
        partials_all = resident.tile([128, 16, G], F32)

        # ================= phase 1 =================
        att_pool = ctx.enter_context(tc.tile_pool(name="att_ps", bufs=2, space="PSUM"))
        xt_pool = ctx.enter_context(tc.tile_pool(name="xt_ps", bufs=2, space="PSUM"))
        pq_pool = ctx.enter_context(tc.tile_pool(name="pq_ps", bufs=1, space="PSUM"))
        t3_pool = ctx.enter_context(tc.tile_pool(name="t3_ps", bufs=1, space="PSUM"))
        t3bank = t3_pool.tile([128, 512], F32, name="t3bank")
        att_hist = []   # last two groups' att psum tiles (full banks)
        xtp_hist = []   # last two groups' xtp psum tile pairs
        if True:
            pq = pq_pool.tile([G, 2 * C], F32)   # [P^T | Q^T]

            def emit_pq(gg, x2tile, split=False):
                """P/Q matmuls for group gg (deferred one group).

                split=True emits all P matmuls before the Qs: the Ps only
                need x^T, so they keep PE busy while DVE finishes x2s."""
                order = ([(j, 0) for j in range(JJ)] + [(j, 1) for j in range(JJ)]
                         ) if split else [(j, k) for j in range(JJ) for k in (0, 1)]
                for j, k in order:
                    t = gg * JJ + j
                    at8 = a_t[:, 8 * t:8 * (t + 1)]
                    if k == 0:
                        nc.tensor.matmul(pq[:, 0:C], lhsT=at8,
                                         rhs=xtr[:, gg % 2, j * TP:(j + 1) * TP],
                                         start=(t == 0), stop=(t == ntiles - 1),
                                         skip_group_check=True)
                    else:
                        nc.tensor.matmul(pq[:, C:2 * C], lhsT=at8,
                                         rhs=x2tile[:, j * TP:(j + 1) * TP],
                                         start=False, stop=(t == ntiles - 1),
                                         skip_group_check=True)

            def emit_aT(gg, c0bank):
                """Transpose group gg's a^T tiles to group-major and copy to
                ags_gm (deferred one group). Chunk 0 reuses group gg's att
                bank (free after exp), chunk 1 the spare bank."""
                for c2 in range(2):
                    bank = c0bank if c2 == 0 else t3bank
                    view = bank[0:8, :].bitcast(F16).rearrange(
                        "p (r t) -> p r t", t=TP)
                    for r in range(8):
                        ti = (gg * 2 + c2) * 8 + r
                        nc.tensor.matmul(view[:, r, :],
                                         lhsT=a_t[:, 8 * ti:8 * (ti + 1)],
                                         rhs=ident[:], is_transpose=True,
                                         start=(r == 0), stop=(r == 7),
                                         skip_group_check=True)
                    flat = bank[0:8, :].bitcast(F16)
                    n1 = gg * GRP + c2 * 1024
                    if c2 == 0:
                        nc.scalar.copy(ags_gm[:, n1:n1 + 1024], flat[:, 0:1024])
                    else:
                        nc.vector.tensor_copy(ags_gm[:, n1:n1 + 1024],
                                              flat[:, 0:1024])

            prev = None  # (group idx, x2 tile)
            dmajunk = stats.tile([1, 4], F16)
            for g in range(ngrp):
                n0 = g * GRP
                if g == 0:
                    xs = xs0
                else:
                    xs = xstage.tile([C, GRP], F32)
                    if g >= 2:
                        # SP-queue clock sync: observe ACT past xc-copy(g-2) so
                        # the bulk DMA below only needs its DMAHW (slot WAW) wait.
                        nc.sync.dma_start(dmajunk[:], xc[0:1, (g - 2) * GRP:(g - 2) * GRP + 4])
                    nc.sync.dma_start(xs[:], x_ext[:, n0:n0 + GRP])
                if g == 0:
                    nc.vector.tensor_copy(xc[:, 0:GRP // 2], xs[:, 0:GRP // 2])
                    nc.vector.tensor_copy(xc[:, GRP // 2:GRP],
                                          xs[:, GRP // 2:GRP])
                else:
                    xcs = xc[:, n0:n0 + GRP]
                    nc.vector.tensor_copy(xcs, xs[:])

                att_bank = att_pool.tile([128, 512], F32)
                att_ps = att_bank[:, 0:JJ * G].rearrange("p (j g) -> p j g", g=G)
                xtp = [xt_pool.tile([128, 8 * TP], F16, name=f"xtp{h}", tag=f"xtp{h}")
                       for h in range(2)]
                att_hist.append(att_bank)
                xtp_hist.append(xtp)
                att_hist = att_hist[-2:]
                xtp_hist = xtp_hist[-2:]
                for j in range(JJ):
                    xcj = xc[:, n0 + j * TP:n0 + (j + 1) * TP]
                    nc.tensor.matmul(
                        xtp[j // 8][:, (j % 8) * TP:(j % 8 + 1) * TP], lhsT=xcj,
                        rhs=ident[:], is_transpose=True,
                        start=(j % 8 == 0), stop=(j % 8 == 7), skip_group_check=True)
                    nc.tensor.matmul(
                        att_ps[:, j, :], lhsT=xcj, rhs=wT[:],
                        start=(j == 0), stop=(j == JJ - 1) and not fold_bias,
                        skip_group_check=True)
                if fold_bias:
                    nc.tensor.matmul(
                        att_bank[:, 0:JJ * G],
                        lhsT=ones_col[:], rhs=b_rep[:].rearrange("p j g -> p (j g)"),
                        start=False, stop=True, skip_group_check=True)

                # previous group's P/Q + a-transposes now that this group's
                # ACT wait is in place
                if prev is not None:
                    emit_pq(*prev)
                    emit_aT(prev[0], att_hist[0])
                    # tiny ACT read of the previous group's last DVE output:
                    # advances ACT's observed DVE clock so exp below doesn't
                    # need a third (DVE WAR) wait slot.
                    sync_junk = stats.tile([128, 1], F16, tag="sync_junk")
                    nc.scalar.copy(sync_junk[:], prev[1][:, 0:1])

                e = estage.tile([128, JJ, G], F32)
                nc.scalar.activation(e[:], att_ps, AF.Exp)
                z = zstage.tile([128, JJ], F32)
                nc.vector.tensor_reduce(z[:], e[:], axis=AX.X, op=OP.add)
                rz = zstage.tile([128, JJ], F32)
                nc.vector.reciprocal(rz[:], z[:])
                atG = a_t[:, g * (GRP // 16):(g + 1) * (GRP // 16)].rearrange(
                    "p (j g) -> p j g", g=G)
                nc.vector.tensor_tensor(atG, e[:], bcast_last(rz[:], G), op=OP.mult)
                nc.vector.tensor_reduce(partials_all[:, g, :],
                                        atG.rearrange("p j g -> p g j"),
                                        axis=AX.X, op=OP.add)

                for h in range(2):
                    nc.scalar.copy(xtr[:, g % 2, h * 8 * TP:(h + 1) * 8 * TP],
                                   xtp[h][:])
                x2s = x2stage.tile([128, GRP], F16)
                nc.vector.tensor_tensor(x2s[:], xtr[:, g % 2, :],
                                        xtr[:, g % 2, :], op=OP.mult)
                prev = (g, x2s)

            emit_pq(*prev, split=True)
            emit_aT(prev[0], att_hist[1])

            NCH = 1024
            HC = 512
            NCC = N // NCH
            if len(att_hist) == 1:
                att_hist = [att_hist[0], att_hist[0]]
                xtp_hist = [xtp_hist[0], xtp_hist[0]]

            # PE filler during the serial stats chain: re-transpose a_t tiles
            # into the (idle until A(0)) a1/a2 banks. Keeping the tensor
            # engine busy here stops its clock from dropping to the half-rate
            # p-state, which it would otherwise keep for all of phase 2.
            def emit_fill(bank_tile, njunk):
                fill = bank_tile[:].bitcast(F16).rearrange(
                    "p (r t) -> p r t", t=TP)
                for r in range(njunk):
                    nc.tensor.matmul(fill[0:8, r % 8, :],
                                     lhsT=a_t[:, 8 * r:8 * (r + 1)],
                                     rhs=ident[:], is_transpose=True,
                                     start=(r == 0), stop=(r == njunk - 1),
                                     skip_group_check=True)

            emit_fill(xtp_hist[0][0], 24)

            # ================= phase 1.5: statistics =================
            partials = stats.tile([128, G], F32)
            nc.vector.tensor_reduce(
                partials[:], partials_all[:].rearrange("p t g -> p g t"),
                axis=AX.X, op=OP.add)
            if True:
                # s_ps borrows a corner of an att bank (its group-major copy
                # is already done; phase 2 overwrites it later in PE order).
                s_ps = att_hist[0][0:G, 0:1]
                nc.tensor.matmul(s_ps, lhsT=partials[:], rhs=onesf32[:],
                                 start=True, stop=True, skip_group_check=True)
                s_eps = stats.tile([G, 1], F32)
                nc.vector.tensor_scalar_add(s_eps[:], s_ps, A_NORM_EPS)
                sden = stats.tile([G, 1], F32)
                nc.vector.reciprocal(sden[:], s_eps[:])
                T = stats.tile([G, 1], F32)
                nc.vector.tensor_tensor(T[:], s_ps, sden[:], op=OP.mult)
                meanT = stats.tile([G, C], F32)
                nc.vector.tensor_scalar_mul(meanT[:], pq[:, 0:C], sden[:])
                m2T = stats.tile([G, C], F32)
                nc.vector.tensor_scalar_mul(m2T[:], pq[:, C:2 * C], sden[:])
            u = stats.tile([G, 1], F32)
            nc.vector.tensor_scalar(u[:], T[:], -1.0, 2.0, op0=OP.mult, op1=OP.add)
            meansq = stats.tile([G, C], F32)
            nc.vector.tensor_tensor(meansq[:], meanT[:], meanT[:], op=OP.mult)
            tmpv = stats.tile([G, C], F32)
            nc.vector.tensor_scalar_mul(tmpv[:], meansq[:], u[:])
            varT = stats.tile([G, C], F32)
            nc.vector.tensor_tensor(varT[:], m2T[:], tmpv[:], op=OP.subtract)
            lnv = stats.tile([G, C], F32)
            nc.scalar.activation(lnv[:], varT[:], AF.Ln, bias=eps_t[:])
            invT = stats.tile([G, C], F32)
            nc.scalar.activation(invT[:], lnv[:], AF.Exp, scale=-0.5)
            Ff = stats.tile([G, C], F32)
            nc.vector.tensor_tensor(Ff[:], meanT[:], invT[:], op=OP.mult)
            # E/F land on ACT so A-matmuls wait on ACT alone
            E = stats.tile([G, C], F16)
            nc.scalar.copy(E[:], invT[:])
            F = stats.tile([G, C], F16)
            nc.scalar.copy(F[:], Ff[:])
            # second PE filler while the DVE/ACT stats chain drains
            emit_fill(xtp_hist[1][0], 40)

        # ================= phase 2: apply =================
        # Pure feed-forward pipeline, matmul-only on PE: A1/A2 into depth-2
        # PSUM banks, ACT stages A1 to SBUF f16 right behind the PE, DVE
        # combines (f16 mult + psum-f32 subtract) one beat back, DMA streams
        # out. Every cross-engine edge has >= 1 iteration of slack.
        tm_db = [tmp_pool.tile([C, HC], F16, name=f"tm{i}", tag=f"tm{i}")
                 for i in range(2)]
        os_db = [opool.tile([C, NCH], F16, name=f"os{i}", tag=f"os{i}")
                 for i in range(3)]
        a1_bk = [[xtp_hist[0][0][:].bitcast(F32), xtp_hist[0][1][:].bitcast(F32)],
                 [att_hist[0][:], att_hist[1][:]]]
        a2_bk = [[xtp_hist[1][0][:].bitcast(F32), xtp_hist[1][1][:].bitcast(F32)],
                 [t3bank[:], t3bank[:]]]
        a1s_db = [[a12_sbp.tile([C, HC], F16, name=f"a1s{h}{i}",
                                tag=f"a1s{h}{i}") for i in range(2)]
                  for h in range(2)]
        for cc in range(NCC):
            n0 = cc * NCH
            ags = ags_gm[:, n0:n0 + NCH]
            os = os_db[cc % 3]
            a1b = [a1_bk[0][cc % 2], a1_bk[1][cc % 2]]
            a2b = [a2_bk[0][cc % 2], a2_bk[1][cc % 2]]
            for h in range(2):
                m0 = h * HC
                nc.tensor.matmul(a1b[h], lhsT=E[:], rhs=ags[:, m0:m0 + HC],
                                 start=True, stop=True, skip_group_check=True)
                nc.tensor.matmul(a2b[h], lhsT=F[:], rhs=ags[:, m0:m0 + HC],
                                 start=True, stop=True, skip_group_check=True)
            for h in range(2):
                nc.scalar.copy(a1s_db[h][cc % 2][:], a1b[h])
            for h in range(2):
                m0 = h * HC
                tm = tm_db[h]
                # h=1 mult runs on gpsimd (SBUF-only operands) so DVE's
                # per-chunk load stays safely below the PE's
                eng = nc.vector if h == 0 else nc.gpsimd
                eng.tensor_tensor(tm[:], xc[:, n0 + m0:n0 + m0 + HC],
                                  a1s_db[h][cc % 2][:], op=OP.mult)
                nc.vector.tensor_tensor(os[:, m0:m0 + HC], tm[:], a2b[h],
                                        op=OP.subtract)
            nc.sync.dma_start(out_ext[:, n0:n0 + NCH], os[:])

    nc.compile()
    return nc


_CACHED_NC = None


def kernel(x, conv_w, conv_b):
    global _CACHED_NC
    x = np.asarray(x)
    conv_w = np.ascontiguousarray(conv_w, dtype=np.float32)
    conv_b = np.asarray(conv_b, dtype=np.float32)
    b, c, n = x.shape[0], x.shape[1], x.shape[2]
    if _CACHED_NC is None:
        _CACHED_NC = build_nc(N=n)
    nc = _CACHED_NC

    in_maps = [
        {
            "x": np.ascontiguousarray(x[i, :, :, 0], dtype=np.float32),
            "conv_w": conv_w,
            "conv_b": conv_b.reshape(1, -1),
        }
        for i in range(b)
    ]
    res = run_bass_kernel_spmd(nc, in_maps, core_ids=list(range(b)))
    out = np.stack([res.results[i]["out"] for i in range(b)])[..., None]
    return out.astype(np.float32)


# revision 35
# speedup vs baseline: 1.0079x; 1.0079x over previous
"""ACN2d multi-branch attentive normalization on 8 TRN2 NeuronCores.

Sharding: data-parallel over batch B (8 samples -> 8 cores), no collectives.
Per core, a two-phase Bass/Tile kernel:
  phase 1 (point-major): PE transposes x to fp16 point-major tiles and
    computes att^T; softmax over the G=8 groups is a free-dim reduce on DVE;
    weighted moments P = x a^T, Q = x^2 a^T and s = sum_n a accumulate on PE.
    a^T is also transposed back to group-major (ags_gm) here, using the
    otherwise-idle PE/ACT/DVE slack, so phase 2 is matmul-only.
  phase 2 (channel-major): A1/A2 are K=8 matmuls against inv_std^T /
    (mean inv_std)^T streaming group-major a; ACT stages A1 to SBUF f16 and
    DVE combines out = x*A1 - A2; results stream to HBM as f16.
The tensor-engine clock drops to a half-rate p-state on any long idle and
needs ~3us of continuous work to recover, so the emission order keeps PE
fed across the stats bubble (filler transposes) and phase 2 keeps >=1
iteration of slack on every cross-engine dependency.
"""

import numpy as np

from concourse.bass_utils import run_bass_kernel_spmd


from contextlib import ExitStack

import concourse.bass as bass
import concourse.bacc as bacc
import concourse.tile as tile
from concourse import mybir
from concourse.masks import make_identity

F32 = mybir.dt.float32
F16 = mybir.dt.float16
AF = mybir.ActivationFunctionType
OP = mybir.AluOpType
AX = mybir.AxisListType

EPS = 1e-3
A_NORM_EPS = 1e-8


def bcast_last(ap, n):
    return bass.AP(tensor=ap.tensor, offset=ap.offset, ap=list(ap.ap) + [[0, n]])


def build_nc(N=32768, fold_bias=True):
    C, G = 128, 8
    TP = 128
    GRP = 2048
    JJ = GRP // TP                # tiles per group = 16
    ntiles = N // TP
    ngrp = N // GRP
    assert N % GRP == 0

    nc = bacc.Bacc("TRN2", target_bir_lowering=False, debug=False)
    x_ext = nc.declare_dram_parameter("x", [C, N], F32, isOutput=False).ap()
    w_ext = nc.declare_dram_parameter("conv_w", [G, C], F32, isOutput=False).ap()
    b_ext = nc.declare_dram_parameter("conv_b", [1, G], F32, isOutput=False).ap()
    out_ext = nc.declare_dram_parameter("out", [C, N], F16, isOutput=True).ap()

    with tile.TileContext(nc) as tc, ExitStack() as ctx:
        consts = ctx.enter_context(tc.tile_pool(name="consts", bufs=1))
        resident = ctx.enter_context(tc.tile_pool(name="resident", bufs=1))
        stats = ctx.enter_context(tc.tile_pool(name="stats", bufs=1))
        # SBUF staging pools stay open for the whole kernel so later pools
        # never inherit WAW deps on the DMA lanes that filled earlier ones.
        xstage = ctx.enter_context(tc.tile_pool(name="xstage", bufs=2))
        x2stage = ctx.enter_context(tc.tile_pool(name="x2stage", bufs=2))
        estage = ctx.enter_context(tc.tile_pool(name="estage", bufs=2))
        zstage = ctx.enter_context(tc.tile_pool(name="zstage", bufs=2))
        a12_sbp = ctx.enter_context(tc.tile_pool(name="a12_sb", bufs=1))
        tmp_pool = ctx.enter_context(tc.tile_pool(name="tmp_sb", bufs=2))
        opool = ctx.enter_context(tc.tile_pool(name="ostage", bufs=1))
        xs0pool = ctx.enter_context(tc.tile_pool(name="xs0p", bufs=1))

        # first x chunk: issue its load before anything else, split in two
        # so the first cast/transposes start after half the transfer
        xs0 = xs0pool.tile([C, GRP], F32, name="xs0")
        nc.sync.dma_start(xs0[:, 0:GRP // 2], x_ext[:, 0:GRP // 2])
        nc.sync.dma_start(xs0[:, GRP // 2:GRP], x_ext[:, GRP // 2:GRP])

        # ---- constants: everything the PE touches funnels through gpsimd ----
        ident = consts.tile([128, 128], F16)
        make_identity(nc, ident)
        ident8 = consts.tile([G, G], F32)
        make_identity(nc, ident8)
        onesf32 = consts.tile([128, 1], F32)
        nc.vector.memset(onesf32, 1.0)
        eps_t = consts.tile([G, 1], F32)
        nc.vector.memset(eps_t, EPS)
        w_sb = consts.tile([G, C], F32)
        nc.sync.dma_start(w_sb[:], w_ext)
        b_row = consts.tile([1, G], F32)
        nc.sync.dma_start(b_row[:], b_ext)
        ones_col = consts.tile([1, 128], F16)
        nc.gpsimd.memset(ones_col, 1.0)
        b_rep = consts.tile([1, JJ, G], F16)
        nc.gpsimd.tensor_copy(
            b_rep[:],
            bass.AP(tensor=b_row[:].tensor, offset=b_row[:].offset,
                    ap=[b_row[:].ap[0], [0, JJ], b_row[:].ap[1]]))
        # bounce conv_w through gpsimd so the wT transpose waits on Pool only
        w_g = consts.tile([G, C], F32)
        nc.gpsimd.tensor_copy(w_g[:], w_sb[:])

        with tc.tile_pool(name="ph0psum", bufs=1, space="PSUM") as ph0psum:
            wT_ps = ph0psum.tile([C, G], F32)
            nc.tensor.transpose(wT_ps[:], w_g[:], ident8[:])
            wT = consts.tile([C, G], F16)
            nc.scalar.copy(wT[:], wT_ps[:])

        # ---- residents ----
        xc = resident.tile([C, N], F16)
        # x^T ring buffer: only the previous group's transpose is ever read
        xtr = resident.tile([128, 2, GRP], F16)
        a_t = resident.tile([128, N // 16], F16)
        # group-major a, built incrementally during phase 1
        ags_gm = resident.tile([G, N], F16)
        # per-group partial sums of a (reduced over tiles); folded into a
        # single cheap reduce at stats time instead of one 3.8us monster.
        partials_all = resident.tile([128, 16, G], F32)

        # ================= phase 1 =================
        att_pool = ctx.enter_context(tc.tile_pool(name="att_ps", bufs=2, space="PSUM"))
        xt_pool = ctx.enter_context(tc.tile_pool(name="xt_ps", bufs=2, space="PSUM"))
        pq_pool = ctx.enter_context(tc.tile_pool(name="pq_ps", bufs=1, space="PSUM"))
        t3_pool = ctx.enter_context(tc.tile_pool(name="t3_ps", bufs=1, space="PSUM"))
        t3bank = t3_pool.tile([128, 512], F32, name="t3bank")
        att_hist = []   # last two groups' att psum tiles (full banks)
        xtp_hist = []   # last two groups' xtp psum tile pairs
        if True:
            pq = pq_pool.tile([G, 2 * C], F32)   # [P^T | Q^T]

            def emit_pq(gg, x2tile, split=False):
                """P/Q matmuls for group gg (deferred one group).

                split=True emits all P matmuls before the Qs: the Ps only
                need x^T, so they keep PE busy while DVE finishes x2s."""
                order = ([(j, 0) for j in range(JJ)] + [(j, 1) for j in range(JJ)]
                         ) if split else [(j, k) for j in range(JJ) for k in (0, 1)]
                for j, k in order:
                    t = gg * JJ + j
                    at8 = a_t[:, 8 * t:8 * (t + 1)]
                    if k == 0:
                        nc.tensor.matmul(pq[:, 0:C], lhsT=at8,
                                         rhs=xtr[:, gg % 2, j * TP:(j + 1) * TP],
                                         start=(t == 0), stop=(t == ntiles - 1),
                                         skip_group_check=True)
                    else:
                        nc.tensor.matmul(pq[:, C:2 * C], lhsT=at8,
                                         rhs=x2tile[:, j * TP:(j + 1) * TP],
                                         start=False, stop=(t == ntiles - 1),
                                         skip_group_check=True)

            def emit_aT(gg, c0bank):
                """Transpose group gg's a^T tiles to group-major and copy to
                ags_gm (deferred one group). Chunk 0 reuses group gg's att
                bank (free after exp), chunk 1 the spare bank."""
                for c2 in range(2):
                    bank = c0bank if c2 == 0 else t3bank
                    view = bank[0:8, :].bitcast(F16).rearrange(
                        "p (r t) -> p r t", t=TP)
                    for r in range(8):
                        ti = (gg * 2 + c2) * 8 + r
                        nc.tensor.matmul(view[:, r, :],
                                         lhsT=a_t[:, 8 * ti:8 * (ti + 1)],
                                         rhs=ident[:], is_transpose=True,
                                         start=(r == 0), stop=(r == 7),
                                         skip_group_check=True)
                    flat = bank[0:8, :].bitcast(F16)
                    n1 = gg * GRP + c2 * 1024
                    if c2 == 0:
                        nc.scalar.copy(ags_gm[:, n1:n1 + 1024], flat[:, 0:1024])
                    else:
                        nc.vector.tensor_copy(ags_gm[:, n1:n1 + 1024],
                                              flat[:, 0:1024])

            prev = None  # (group idx, x2 tile)
            dmajunk = stats.tile([1, 4], F16)
            for g in range(ngrp):
                n0 = g * GRP
                if g == 0:
                    xs = xs0
                else:
                    xs = xstage.tile([C, GRP], F32)
                    if g >= 2:
                        # SP-queue clock sync: observe ACT past xc-copy(g-2) so
                        # the bulk DMA below only needs its DMAHW (slot WAW) wait.
                        nc.sync.dma_start(dmajunk[:], xc[0:1, (g - 2) * GRP:(g - 2) * GRP + 4])
                    nc.sync.dma_start(xs[:], x_ext[:, n0:n0 + GRP])
                if g == 0:
                    nc.vector.tensor_copy(xc[:, 0:GRP // 2], xs[:, 0:GRP // 2])
                    nc.vector.tensor_copy(xc[:, GRP // 2:GRP],
                                          xs[:, GRP // 2:GRP])
                else:
                    xcs = xc[:, n0:n0 + GRP]
                    nc.vector.tensor_copy(xcs, xs[:])

                att_bank = att_pool.tile([128, 512], F32)
                att_ps = att_bank[:, 0:JJ * G].rearrange("p (j g) -> p j g", g=G)
                xtp = [xt_pool.tile([128, 8 * TP], F16, name=f"xtp{h}", tag=f"xtp{h}")
                       for h in range(2)]
                att_hist.append(att_bank)
                xtp_hist.append(xtp)
                att_hist = att_hist[-2:]
                xtp_hist = xtp_hist[-2:]
                for j in range(JJ):
                    xcj = xc[:, n0 + j * TP:n0 + (j + 1) * TP]
                    nc.tensor.matmul(
                        xtp[j // 8][:, (j % 8) * TP:(j % 8 + 1) * TP], lhsT=xcj,
                        rhs=ident[:], is_transpose=True,
                        start=(j % 8 == 0), stop=(j % 8 == 7), skip_group_check=True)
                    nc.tensor.matmul(
                        att_ps[:, j, :], lhsT=xcj, rhs=wT[:],
                        start=(j == 0), stop=(j == JJ - 1) and not fold_bias,
                        skip_group_check=True)
                if fold_bias:
                    nc.tensor.matmul(
                        att_bank[:, 0:JJ * G],
                        lhsT=ones_col[:], rhs=b_rep[:].rearrange("p j g -> p (j g)"),
                        start=False, stop=True, skip_group_check=True)

                # previous group's P/Q + a-transposes now that this group's
                # ACT wait is in place
                if prev is not None:
                    emit_pq(*prev)
                    emit_aT(prev[0], att_hist[0])
                    # tiny ACT read of the previous group's last DVE output:
                    # advances ACT's observed DVE clock so exp below doesn't
                    # need a third (DVE WAR) wait slot.
                    sync_junk = stats.tile([128, 1], F16, tag="sync_junk")
                    nc.scalar.copy(sync_junk[:], prev[1][:, 0:1])

                e = estage.tile([128, JJ, G], F32)
                nc.scalar.activation(e[:], att_ps, AF.Exp)
                z = zstage.tile([128, JJ], F32)
                nc.vector.tensor_reduce(z[:], e[:], axis=AX.X, op=OP.add)
                rz = zstage.tile([128, JJ], F32)
                nc.vector.reciprocal(rz[:], z[:])
                atG = a_t[:, g * (GRP // 16):(g + 1) * (GRP // 16)].rearrange(
                    "p (j g) -> p j g", g=G)
                nc.vector.tensor_tensor(atG, e[:], bcast_last(rz[:], G), op=OP.mult)
                nc.vector.tensor_reduce(partials_all[:, g, :],
                                        atG.rearrange("p j g -> p g j"),
                                        axis=AX.X, op=OP.add)

                for h in range(2):
                    nc.scalar.copy(xtr[:, g % 2, h * 8 * TP:(h + 1) * 8 * TP],
                                   xtp[h][:])
                x2s = x2stage.tile([128, GRP], F16)
                nc.vector.tensor_tensor(x2s[:], xtr[:, g % 2, :],
                                        xtr[:, g % 2, :], op=OP.mult)
                prev = (g, x2s)

            emit_pq(*prev, split=True)
            emit_aT(prev[0], att_hist[1])

            NCH = 1024
            HC = 512
            NCC = N // NCH
            if len(att_hist) == 1:
                att_hist = [att_hist[0], att_hist[0]]
                xtp_hist = [xtp_hist[0], xtp_hist[0]]

            # PE filler during the serial stats chain: re-transpose a_t tiles
            # into the (idle until A(0)) a1/a2 banks. Keeping the tensor
            # engine busy here stops its clock from dropping to the half-rate
            # p-state, which it would otherwise keep for all of phase 2.
            def emit_fill(bank_tile, njunk):
                fill = bank_tile[:].bitcast(F16).rearrange(
                    "p (r t) -> p r t", t=TP)
                for r in range(njunk):
                    nc.tensor.matmul(fill[0:8, r % 8, :],
                                     lhsT=a_t[:, 8 * r:8 * (r + 1)],
                                     rhs=ident[:], is_transpose=True,
                                     start=(r == 0), stop=(r == njunk - 1),
                                     skip_group_check=True)

            emit_fill(xtp_hist[0][0], 24)

            # ================= phase 1.5: statistics =================
            partials = stats.tile([128, G], F32)
            nc.vector.tensor_reduce(
                partials[:], partials_all[:].rearrange("p t g -> p g t"),
                axis=AX.X, op=OP.add)
            if True:
                # s_ps borrows a corner of an att bank (its group-major copy
                # is already done; phase 2 overwrites it later in PE order).
                s_ps = att_hist[0][0:G, 0:1]
                nc.tensor.matmul(s_ps, lhsT=partials[:], rhs=onesf32[:],
                                 start=True, stop=True, skip_group_check=True)
                s_eps = stats.tile([G, 1], F32)
                nc.vector.tensor_scalar_add(s_eps[:], s_ps, A_NORM_EPS)
                sden = stats.tile([G, 1], F32)
                nc.vector.reciprocal(sden[:], s_eps[:])
                T = stats.tile([G, 1], F32)
                nc.vector.tensor_tensor(T[:], s_ps, sden[:], op=OP.mult)
                meanT = stats.tile([G, C], F32)
                nc.vector.tensor_scalar_mul(meanT[:], pq[:, 0:C], sden[:])
                m2T = stats.tile([G, C], F32)
                nc.vector.tensor_scalar_mul(m2T[:], pq[:, C:2 * C], sden[:])
            u = stats.tile([G, 1], F32)
            nc.vector.tensor_scalar(u[:], T[:], -1.0, 2.0, op0=OP.mult, op1=OP.add)
            meansq = stats.tile([G, C], F32)
            nc.vector.tensor_tensor(meansq[:], meanT[:], meanT[:], op=OP.mult)
            tmpv = stats.tile([G, C], F32)
            nc.vector.tensor_scalar_mul(tmpv[:], meansq[:], u[:])
            varT = stats.tile([G, C], F32)
            nc.vector.tensor_tensor(varT[:], m2T[:], tmpv[:], op=OP.subtract)
            lnv = stats.tile([G, C], F32)
            nc.scalar.activation(lnv[:], varT[:], AF.Ln, bias=eps_t[:])
            invT = stats.tile([G, C], F32)
            nc.scalar.activation(invT[:], lnv[:], AF.Exp, scale=-0.5)
            Ff = stats.tile([G, C], F32)
            nc.vector.tensor_tensor(Ff[:], meanT[:], invT[:], op=OP.mult)
            # E/F land on ACT so A-matmuls wait on ACT alone
            E = stats.tile([G, C], F16)
            nc.scalar.copy(E[:], invT[:])
            F = stats.tile([G, C], F16)
            nc.scalar.copy(F[:], Ff[:])
            # second PE filler while the DVE/ACT stats chain drains
            emit_fill(xtp_hist[1][0], 40)

        # ================= phase 2: apply =================
        # Pure feed-forward pipeline, matmul-only on PE: A1/A2 into depth-2
        # PSUM banks, ACT stages A1 to SBUF f16 right behind the PE, DVE
        # combines (f16 mult + psum-f32 subtract) one beat back, DMA streams
        # out. Every cross-engine edge has >= 1 iteration of slack.
        tm_db = [tmp_pool.tile([C, HC], F16, name=f"tm{i}", tag=f"tm{i}")
                 for i in range(2)]
        os_db = [opool.tile([C, NCH], F16, name=f"os{i}", tag=f"os{i}")
                 for i in range(3)]
        a1_bk = [[xtp_hist[0][0][:].bitcast(F32), xtp_hist[0][1][:].bitcast(F32)],
                 [att_hist[0][:], att_hist[1][:]]]
        a2_bk = [[xtp_hist[1][0][:].bitcast(F32), xtp_hist[1][1][:].bitcast(F32)],
                 [t3bank[:], t3bank[:]]]
        a1s_db = [[a12_sbp.tile([C, HC], F16, name=f"a1s{h}{i}",
                                tag=f"a1s{h}{i}") for i in range(2)]
                  for h in range(2)]
        a2s_db = [a12_sbp.tile([C, HC], F16, name=f"a2s{i}", tag=f"a2s{i}")
                  for i in range(2)]
        for cc in range(NCC):
            n0 = cc * NCH
            ags = ags_gm[:, n0:n0 + NCH]
            os = os_db[cc % 3]
            a1b = [a1_bk[0][cc % 2], a1_bk[1][cc % 2]]
            a2b = [a2_bk[0][cc % 2], a2_bk[1][cc % 2]]
            for h in range(2):
                m0 = h * HC
                nc.tensor.matmul(a1b[h], lhsT=E[:], rhs=ags[:, m0:m0 + HC],
                                 start=True, stop=True, skip_group_check=True)
                nc.tensor.matmul(a2b[h], lhsT=F[:], rhs=ags[:, m0:m0 + HC],
                                 start=True, stop=True, skip_group_check=True)
            for h in range(2):
                nc.scalar.copy(a1s_db[h][cc % 2][:], a1b[h])
            # a2 h=0 also staged to SBUF f16 by ACT so that subtract runs at
            # the f16 rate on DVE; h=1 subtract reads its PSUM bank directly.
            nc.scalar.copy(a2s_db[cc % 2][:], a2b[0])
            for h in range(2):
                m0 = h * HC
                tm = tm_db[h]
                nc.vector.tensor_tensor(tm[:], xc[:, n0 + m0:n0 + m0 + HC],
                                        a1s_db[h][cc % 2][:], op=OP.mult)
                sub_src = a2s_db[cc % 2][:] if h == 0 else a2b[h]
                nc.vector.tensor_tensor(os[:, m0:m0 + HC], tm[:], sub_src,
                                        op=OP.subtract)
            nc.sync.dma_start(out_ext[:, n0:n0 + NCH], os[:])

    nc.compile()
    return nc


_CACHED_NC = None


def kernel(x, conv_w, conv_b):
    global _CACHED_NC
    x = np.asarray(x)
    conv_w = np.ascontiguousarray(conv_w, dtype=np.float32)
    conv_b = np.asarray(conv_b, dtype=np.float32)
    b, c, n = x.shape[0], x.shape[1], x.shape[2]
    if _CACHED_NC is None:
        _CACHED_NC = build_nc(N=n)
    nc = _CACHED_NC

    in_maps = [
        {
            "x": np.ascontiguousarray(x[i, :, :, 0], dtype=np.float32),
            "conv_w": conv_w,
            "conv_b": conv_b.reshape(1, -1),
        }
        for i in range(b)
    ]
    res = run_bass_kernel_spmd(nc, in_maps, core_ids=list(range(b)))
    out = np.stack([res.results[i]["out"] for i in range(b)])[..., None]
    return out.astype(np.float32)


# revision 39
# speedup vs baseline: 1.0427x; 1.0345x over previous
"""ACN2d multi-branch attentive normalization on 8 TRN2 NeuronCores.

Sharding: data-parallel over batch B (8 samples -> 8 cores), no collectives.
Per core, a two-phase Bass/Tile kernel:
  phase 1 (point-major): PE transposes x to fp16 point-major tiles and
    computes att^T; softmax over the G=8 groups is a free-dim reduce on DVE;
    weighted moments P = x a^T, Q = x^2 a^T and s = sum_n a accumulate on PE.
    a^T is also transposed back to group-major (ags_gm) here, using the
    otherwise-idle PE/ACT/DVE slack, so phase 2 is matmul-only.
  phase 2 (channel-major): A1/A2 are K=8 matmuls against inv_std^T /
    (mean inv_std)^T streaming group-major a; ACT stages A1 to SBUF f16 and
    DVE combines out = x*A1 - A2; results stream to HBM as f16.
The tensor-engine clock drops to a half-rate p-state on any long idle and
needs ~3us of continuous work to recover, so the emission order keeps PE
fed across the stats bubble (filler transposes) and phase 2 keeps >=1
iteration of slack on every cross-engine dependency.
"""

import numpy as np

from concourse.bass_utils import run_bass_kernel_spmd


from contextlib import ExitStack

import concourse.bass as bass
import concourse.bacc as bacc
import concourse.tile as tile
from concourse import mybir
from concourse.masks import make_identity

F32 = mybir.dt.float32
F16 = mybir.dt.float16
AF = mybir.ActivationFunctionType
OP = mybir.AluOpType
AX = mybir.AxisListType

EPS = 1e-3
A_NORM_EPS = 1e-8


def bcast_last(ap, n):
    return bass.AP(tensor=ap.tensor, offset=ap.offset, ap=list(ap.ap) + [[0, n]])


def build_nc(N=32768, fold_bias=True):
    C, G = 128, 8
    TP = 128
    GRP = 2048
    JJ = GRP // TP                # tiles per group = 16
    ntiles = N // TP
    ngrp = N // GRP
    assert N % GRP == 0

    nc = bacc.Bacc("TRN2", target_bir_lowering=False, debug=False)
    x_ext = nc.declare_dram_parameter("x", [C, N], F32, isOutput=False).ap()
    w_ext = nc.declare_dram_parameter("conv_w", [G, C], F32, isOutput=False).ap()
    b_ext = nc.declare_dram_parameter("conv_b", [1, G], F32, isOutput=False).ap()
    out_ext = nc.declare_dram_parameter("out", [C, N], F16, isOutput=True).ap()

    with tile.TileContext(nc) as tc, ExitStack() as ctx:
        consts = ctx.enter_context(tc.tile_pool(name="consts", bufs=1))
        resident = ctx.enter_context(tc.tile_pool(name="resident", bufs=1))
        stats = ctx.enter_context(tc.tile_pool(name="stats", bufs=1))
        # SBUF staging pools stay open for the whole kernel so later pools
        # never inherit WAW deps on the DMA lanes that filled earlier ones.
        xstage = ctx.enter_context(tc.tile_pool(name="xstage", bufs=2))
        x2stage = ctx.enter_context(tc.tile_pool(name="x2stage", bufs=2))
        estage = ctx.enter_context(tc.tile_pool(name="estage", bufs=2))
        zstage = ctx.enter_context(tc.tile_pool(name="zstage", bufs=2))
        a12_sbp = ctx.enter_context(tc.tile_pool(name="a12_sb", bufs=1))
        tmp_pool = ctx.enter_context(tc.tile_pool(name="tmp_sb", bufs=2))
        opool = ctx.enter_context(tc.tile_pool(name="ostage", bufs=1))
        xs0pool = ctx.enter_context(tc.tile_pool(name="xs0p", bufs=1))

        # first x chunk: issue its load before anything else so phase 1
        # starts as early as possible
        xs0 = xs0pool.tile([C, GRP], F32, name="xs0")
        nc.sync.dma_start(xs0[:], x_ext[:, 0:GRP])

        # ---- constants: everything the PE touches funnels through gpsimd ----
        ident = consts.tile([128, 128], F16)
        make_identity(nc, ident)
        ident8 = consts.tile([G, G], F32)
        make_identity(nc, ident8)
        onesf32 = consts.tile([128, 1], F32)
        nc.vector.memset(onesf32, 1.0)
        eps_t = consts.tile([G, 1], F32)
        nc.vector.memset(eps_t, EPS)
        w_sb = consts.tile([G, C], F32)
        nc.sync.dma_start(w_sb[:], w_ext)
        b_row = consts.tile([1, G], F32)
        nc.sync.dma_start(b_row[:], b_ext)
        ones_col = consts.tile([1, 128], F16)
        nc.gpsimd.memset(ones_col, 1.0)
        b_rep = consts.tile([1, JJ, G], F16)
        nc.gpsimd.tensor_copy(
            b_rep[:],
            bass.AP(tensor=b_row[:].tensor, offset=b_row[:].offset,
                    ap=[b_row[:].ap[0], [0, JJ], b_row[:].ap[1]]))
        # bounce conv_w through gpsimd so the wT transpose waits on Pool only
        w_g = consts.tile([G, C], F32)
        nc.gpsimd.tensor_copy(w_g[:], w_sb[:])

        with tc.tile_pool(name="ph0psum", bufs=1, space="PSUM") as ph0psum:
            wT_ps = ph0psum.tile([C, G], F32)
            nc.tensor.transpose(wT_ps[:], w_g[:], ident8[:])
            wT = consts.tile([C, G], F16)
            nc.scalar.copy(wT[:], wT_ps[:])

        # ---- residents ----
        xc = resident.tile([C, N], F16)
        # x^T ring buffer: only the previous group's transpose is ever read
        xtr = resident.tile([128, 2, GRP], F16)
        a_t = resident.tile([128, N // 16], F16)
        # group-major a, built incrementally during phase 1
        ags_gm = resident.tile([G, N], F16)
        # per-group partial sums of a (reduced over tiles); folded into a
        # single cheap reduce at stats time instead of one 3.8us monster.
        partials_all = resident.tile([128, 16, G], F32)

        # ================= phase 1 =================
        att_pool = ctx.enter_context(tc.tile_pool(name="att_ps", bufs=2, space="PSUM"))
        xt_pool = ctx.enter_context(tc.tile_pool(name="xt_ps", bufs=2, space="PSUM"))
        pq_pool = ctx.enter_context(tc.tile_pool(name="pq_ps", bufs=1, space="PSUM"))
        t3_pool = ctx.enter_context(tc.tile_pool(name="t3_ps", bufs=1, space="PSUM"))
        t3bank = t3_pool.tile([128, 512], F32, name="t3bank")
        att_hist = []   # last two groups' att psum tiles (full banks)
        xtp_hist = []   # last two groups' xtp psum tile pairs
        if True:
            pq = pq_pool.tile([G, 2 * C], F32)   # [P^T | Q^T]

            def emit_pq(gg, x2tile, split=False):
                """P/Q matmuls for group gg (deferred one group).

                split=True emits all P matmuls before the Qs: the Ps only
                need x^T, so they keep PE busy while DVE finishes x2s."""
                order = ([(j, 0) for j in range(JJ)] + [(j, 1) for j in range(JJ)]
                         ) if split else [(j, k) for j in range(JJ) for k in (0, 1)]
                for j, k in order:
                    t = gg * JJ + j
                    at8 = a_t[:, 8 * t:8 * (t + 1)]
                    if k == 0:
                        nc.tensor.matmul(pq[:, 0:C], lhsT=at8,
                                         rhs=xtr[:, gg % 2, j * TP:(j + 1) * TP],
                                         start=(t == 0), stop=(t == ntiles - 1),
                                         skip_group_check=True)
                    else:
                        nc.tensor.matmul(pq[:, C:2 * C], lhsT=at8,
                                         rhs=x2tile[:, j * TP:(j + 1) * TP],
                                         start=False, stop=(t == ntiles - 1),
                                         skip_group_check=True)

            def emit_aT(gg, c0bank):
                """Transpose group gg's a^T tiles to group-major and copy to
                ags_gm (deferred one group). Chunk 0 reuses group gg's att
                bank (free after exp), chunk 1 the spare bank."""
                for c2 in range(2):
                    bank = c0bank if c2 == 0 else t3bank
                    view = bank[0:8, :].bitcast(F16).rearrange(
                        "p (r t) -> p r t", t=TP)
                    for r in range(8):
                        ti = (gg * 2 + c2) * 8 + r
                        nc.tensor.matmul(view[:, r, :],
                                         lhsT=a_t[:, 8 * ti:8 * (ti + 1)],
                                         rhs=ident[:], is_transpose=True,
                                         start=(r == 0), stop=(r == 7),
                                         skip_group_check=True)
                    flat = bank[0:8, :].bitcast(F16)
                    n1 = gg * GRP + c2 * 1024
                    if c2 == 0:
                        nc.scalar.copy(ags_gm[:, n1:n1 + 1024], flat[:, 0:1024])
                    else:
                        nc.vector.tensor_copy(ags_gm[:, n1:n1 + 1024],
                                              flat[:, 0:1024])

            prev = None  # (group idx, x2 tile)
            dmajunk = stats.tile([1, 4], F16)
            for g in range(ngrp):
                n0 = g * GRP
                if g == 0:
                    xs = xs0
                else:
                    xs = xstage.tile([C, GRP], F32)
                    if g >= 2:
                        # SP-queue clock sync: observe ACT past xc-copy(g-2) so
                        # the bulk DMA below only needs its DMAHW (slot WAW) wait.
                        nc.sync.dma_start(dmajunk[:], xc[0:1, (g - 2) * GRP:(g - 2) * GRP + 4])
                    nc.sync.dma_start(xs[:], x_ext[:, n0:n0 + GRP])
                xcs = xc[:, n0:n0 + GRP]
                nc.vector.tensor_copy(xcs, xs[:])

                att_bank = att_pool.tile([128, 512], F32)
                att_ps = att_bank[:, 0:JJ * G].rearrange("p (j g) -> p j g", g=G)
                xtp = [xt_pool.tile([128, 8 * TP], F16, name=f"xtp{h}", tag=f"xtp{h}")
                       for h in range(2)]
                att_hist.append(att_bank)
                xtp_hist.append(xtp)
                att_hist = att_hist[-2:]
                xtp_hist = xtp_hist[-2:]
                for j in range(JJ):
                    xcj = xc[:, n0 + j * TP:n0 + (j + 1) * TP]
                    nc.tensor.matmul(
                        xtp[j // 8][:, (j % 8) * TP:(j % 8 + 1) * TP], lhsT=xcj,
                        rhs=ident[:], is_transpose=True,
                        start=(j % 8 == 0), stop=(j % 8 == 7), skip_group_check=True)
                    nc.tensor.matmul(
                        att_ps[:, j, :], lhsT=xcj, rhs=wT[:],
                        start=(j == 0), stop=(j == JJ - 1) and not fold_bias,
                        skip_group_check=True)
                if fold_bias:
                    nc.tensor.matmul(
                        att_bank[:, 0:JJ * G],
                        lhsT=ones_col[:], rhs=b_rep[:].rearrange("p j g -> p (j g)"),
                        start=False, stop=True, skip_group_check=True)

                # previous group's P/Q + a-transposes now that this group's
                # ACT wait is in place
                if prev is not None:
                    emit_pq(*prev)
                    emit_aT(prev[0], att_hist[0])
                    # tiny ACT read of the previous group's last DVE output:
                    # advances ACT's observed DVE clock so exp below doesn't
                    # need a third (DVE WAR) wait slot.
                    sync_junk = stats.tile([128, 1], F16, tag="sync_junk")
                    nc.scalar.copy(sync_junk[:], prev[1][:, 0:1])

                e = estage.tile([128, JJ, G], F32)
                nc.scalar.activation(e[:], att_ps, AF.Exp)
                z = zstage.tile([128, JJ], F32)
                nc.vector.tensor_reduce(z[:], e[:], axis=AX.X, op=OP.add)
                rz = zstage.tile([128, JJ], F32)
                nc.vector.reciprocal(rz[:], z[:])
                atG = a_t[:, g * (GRP // 16):(g + 1) * (GRP // 16)].rearrange(
                    "p (j g) -> p j g", g=G)
                nc.vector.tensor_tensor(atG, e[:], bcast_last(rz[:], G), op=OP.mult)
                nc.vector.tensor_reduce(partials_all[:, g, :],
                                        atG.rearrange("p j g -> p g j"),
                                        axis=AX.X, op=OP.add)

                for h in range(2):
                    nc.scalar.copy(xtr[:, g % 2, h * 8 * TP:(h + 1) * 8 * TP],
                                   xtp[h][:])
                x2s = x2stage.tile([128, GRP], F16)
                nc.vector.tensor_tensor(x2s[:], xtr[:, g % 2, :],
                                        xtr[:, g % 2, :], op=OP.mult)
                prev = (g, x2s)

            emit_pq(*prev, split=True)
            emit_aT(prev[0], att_hist[1])

            NCH = 1024
            HC = 512
            NCC = N // NCH
            if len(att_hist) == 1:
                att_hist = [att_hist[0], att_hist[0]]
                xtp_hist = [xtp_hist[0], xtp_hist[0]]

            # PE filler during the serial stats chain: re-transpose a_t tiles
            # into the (idle until A(0)) a1/a2 banks. Keeping the tensor
            # engine busy here stops its clock from dropping to the half-rate
            # p-state, which it would otherwise keep for all of phase 2.
            def emit_fill(bank_tile, njunk):
                fill = bank_tile[:].bitcast(F16).rearrange(
                    "p (r t) -> p r t", t=TP)
                for r in range(njunk):
                    nc.tensor.matmul(fill[0:8, r % 8, :],
                                     lhsT=a_t[:, 8 * r:8 * (r + 1)],
                                     rhs=ident[:], is_transpose=True,
                                     start=(r == 0), stop=(r == njunk - 1),
                                     skip_group_check=True)

            emit_fill(xtp_hist[0][0], 24)

            # ================= phase 1.5: statistics =================
            partials = stats.tile([128, G], F32)
            nc.vector.tensor_reduce(
                partials[:], partials_all[:].rearrange("p t g -> p g t"),
                axis=AX.X, op=OP.add)
            if True:
                # s_ps borrows a corner of an att bank (its group-major copy
                # is already done; phase 2 overwrites it later in PE order).
                s_ps = att_hist[0][0:G, 0:1]
                nc.tensor.matmul(s_ps, lhsT=partials[:], rhs=onesf32[:],
                                 start=True, stop=True, skip_group_check=True)
                s_eps = stats.tile([G, 1], F32)
                nc.vector.tensor_scalar_add(s_eps[:], s_ps, A_NORM_EPS)
                sden = stats.tile([G, 1], F32)
                nc.vector.reciprocal(sden[:], s_eps[:])
                T = stats.tile([G, 1], F32)
                nc.vector.tensor_tensor(T[:], s_ps, sden[:], op=OP.mult)
                meanT = stats.tile([G, C], F32)
                nc.vector.tensor_scalar_mul(meanT[:], pq[:, 0:C], sden[:])
                m2T = stats.tile([G, C], F32)
                nc.vector.tensor_scalar_mul(m2T[:], pq[:, C:2 * C], sden[:])
            u = stats.tile([G, 1], F32)
            nc.vector.tensor_scalar(u[:], T[:], -1.0, 2.0, op0=OP.mult, op1=OP.add)
            meansq = stats.tile([G, C], F32)
            nc.vector.tensor_tensor(meansq[:], meanT[:], meanT[:], op=OP.mult)
            tmpv = stats.tile([G, C], F32)
            nc.vector.tensor_scalar_mul(tmpv[:], meansq[:], u[:])
            varT = stats.tile([G, C], F32)
            nc.vector.tensor_tensor(varT[:], m2T[:], tmpv[:], op=OP.subtract)
            lnv = stats.tile([G, C], F32)
            nc.scalar.activation(lnv[:], varT[:], AF.Ln, bias=eps_t[:])
            invT = stats.tile([G, C], F32)
            nc.scalar.activation(invT[:], lnv[:], AF.Exp, scale=-0.5)
            Ff = stats.tile([G, C], F32)
            nc.vector.tensor_tensor(Ff[:], meanT[:], invT[:], op=OP.mult)
            # E/F land on ACT so A-matmuls wait on ACT alone
            E = stats.tile([G, C], F16)
            nc.scalar.copy(E[:], invT[:])
            F = stats.tile([G, C], F16)
            nc.scalar.copy(F[:], Ff[:])
            # second PE filler while the DVE/ACT stats chain drains
            emit_fill(xtp_hist[1][0], 40)

        # ================= phase 2: apply =================
        # Pure feed-forward pipeline, matmul-only on PE: A1/A2 into depth-2
        # PSUM banks, ACT stages A1 to SBUF f16 right behind the PE, DVE
        # combines (f16 mult + psum-f32 subtract) one beat back, DMA streams
        # out. Every cross-engine edge has >= 1 iteration of slack.
        tm_db = [tmp_pool.tile([C, HC], F16, name=f"tm{i}", tag=f"tm{i}")
                 for i in range(2)]
        os_db = [opool.tile([C, NCH], F16, name=f"os{i}", tag=f"os{i}")
                 for i in range(3)]
        a1_bk = [[xtp_hist[0][0][:].bitcast(F32), xtp_hist[0][1][:].bitcast(F32)],
                 [att_hist[0][:], att_hist[1][:]]]
        a2_bk = [[xtp_hist[1][0][:].bitcast(F32), xtp_hist[1][1][:].bitcast(F32)],
                 [t3bank[:], t3bank[:]]]
        a1s_db = [[a12_sbp.tile([C, HC], F16, name=f"a1s{h}{i}",
                                tag=f"a1s{h}{i}") for i in range(2)]
                  for h in range(2)]
        for cc in range(NCC):
            n0 = cc * NCH
            ags = ags_gm[:, n0:n0 + NCH]
            os = os_db[cc % 3]
            a1b = [a1_bk[0][cc % 2], a1_bk[1][cc % 2]]
            a2b = [a2_bk[0][cc % 2], a2_bk[1][cc % 2]]
            for h in range(2):
                m0 = h * HC
                nc.tensor.matmul(a1b[h], lhsT=E[:], rhs=ags[:, m0:m0 + HC],
                                 start=True, stop=True, skip_group_check=True)
                nc.tensor.matmul(a2b[h], lhsT=F[:], rhs=ags[:, m0:m0 + HC],
                                 start=True, stop=True, skip_group_check=True)
            for h in range(2):
                nc.scalar.copy(a1s_db[h][cc % 2][:], a1b[h])
            for h in range(2):
                m0 = h * HC
                tm = tm_db[h]
                nc.vector.tensor_tensor(tm[:], xc[:, n0 + m0:n0 + m0 + HC],
                                        a1s_db[h][cc % 2][:], op=OP.mult)
                nc.vector.tensor_tensor(os[:, m0:m0 + HC], tm[:], a2b[h],
                                        op=OP.subtract)
            nc.sync.dma_start(out_ext[:, n0:n0 + NCH], os[:])

    nc.compile()
    return nc


_CACHED_NC = None


def kernel(x, conv_w, conv_b):
    global _CACHED_NC
    x = np.asarray(x)
    conv_w = np.ascontiguousarray(conv_w, dtype=np.float32)
    conv_b = np.asarray(conv_b, dtype=np.float32)
    b, c, n = x.shape[0], x.shape[1], x.shape[2]
    if _CACHED_NC is None:
        _CACHED_NC = build_nc(N=n)
    nc = _CACHED_NC

    in_maps = [
        {
            "x": np.ascontiguousarray(x[i, :, :, 0], dtype=np.float32),
            "conv_w": conv_w,
            "conv_b": conv_b.reshape(1, -1),
        }
        for i in range(b)
    ]
    res = run_bass_kernel_spmd(nc, in_maps, core_ids=list(range(b)))
    out = np.stack([res.results[i]["out"] for i in range(b)])[..., None]
    return out.astype(np.float32)
